# revision 8
# baseline (speedup 1.0000x reference)
"""Trainium2 Bass kernel for nn_Policy_11484742550172.

The reference pads each input channel with 100 zeros on the right and keeps
the last 32 columns — with 100 >= 32 the conv input is exactly zero for any
x, so the network collapses to a weights-only dense chain:

    v1 = relu(conv1_b)                                  [8]
    v2 = relu(sum_k conv2_w[:, :, k] @ v1 + conv2_b)    [16]
    v3 = relu(sum_k conv3_w[:, :, k] @ v2 + conv3_b)    [32]
    v4 = relu(conv4_w[:, :, 0] @ v3 + conv4_b)          [32]
    h   = relu(fc1_w.reshape(128, 32, 30).sum(-1) @ v4 + fc1_b)
    out = softmax(fc2_w @ h + fc2_b)
        = sigmoid([l0 - l1, l1 - l0])   (softmax over 2 = sigmoid of diff)

This is an exact algebraic simplification (conv of zeros = bias), not an
approximation. x and conv1_w never influence the output.

Small weights/biases are host-packed into two tensors (conv-critical part
first so the chain starts as early as possible); fc1_w (99% of the bytes)
ships unmodified in 4 chunks spread over both DGE paths. The conv chain
runs on PE + ScalarE while the DVE does the fc1 group-sum reductions, and
relu/sigmoid/copy all live in one ACT table set (warmed early).

Sharding: the problem is far too small to shard; the kernel is replicated
SPMD on all 8 cores and core 0's output is returned.
"""

import numpy as np

import concourse.bass as bass
import concourse.tile as tile
from concourse import bacc, mybir
from concourse.bass_utils import run_bass_kernel_spmd

N_CORES = 8
F32 = mybir.dt.float32
ALU = mybir.AluOpType
ACT = mybir.ActivationFunctionType
X = mybir.AxisListType.X

_CACHE = {}


def _build():
    nc = bacc.Bacc(
        "TRN2",
        target_bir_lowering=False,
        debug=False,
        num_devices=N_CORES,
        enable_partition_id=False,
    )

    pkad = nc.dram_tensor("pka", [32, 132], F32, kind="ExternalInput")
    pkbd = nc.dram_tensor("pkb", [128, 5], F32, kind="ExternalInput")
    identd = nc.dram_tensor("ident", [128, 128], F32, kind="ExternalInput")
    fw1d = nc.dram_tensor("fc1_w", [128, 960], F32, kind="ExternalInput")
    outd = nc.dram_tensor("out", [1, 2], F32, kind="ExternalOutput")

    with tile.TileContext(nc) as tc:
        with (
            tc.tile_pool(name="sb", bufs=1) as sb,
            tc.tile_pool(name="ps", bufs=1, space="PSUM") as ps,
        ):
            zero = nc.const_aps.aps[(F32, 0.0)]
            one = nc.const_aps.aps[(F32, 1.0)]

            # Warm the sigmoid_and_others ACT table (covers relu/copy/sigmoid)
            # while DMAs are in flight.
            warm = sb.tile([1, 1], F32)
            nc.scalar.activation(warm[:], zero[:1, :1], ACT.Sigmoid)

            # --- loads: conv-critical pack first, fc1_w split over both DGE
            # paths, identity + tail pack wherever there is queue room ---
            pka = sb.tile([32, 132], F32)
            nc.sync.dma_start(pka[:], pkad[:])
            fw1 = sb.tile([128, 960], F32)
            nc.gpsimd.dma_start(fw1[:, 0:240], fw1d[:, 0:240])
            nc.gpsimd.dma_start(fw1[:, 240:480], fw1d[:, 240:480])
            nc.sync.dma_start(fw1[:, 480:720], fw1d[:, 480:720])
            nc.sync.dma_start(fw1[:, 720:960], fw1d[:, 720:960])
            ident = sb.tile([128, 128], F32)
            nc.gpsimd.dma_start(ident[:], identd[:])
            pkb = sb.tile([128, 5], F32)
            nc.sync.dma_start(pkb[:], pkbd[:])

            b1 = pka[0:8, 0:1]
            b2 = pka[0:16, 1:2]
            b3 = pka[0:32, 2:3]
            b4 = pka[0:32, 3:4]
            w2v = pka[0:8, 4:36].rearrange("i (o k) -> i o k", k=2)
            w3v = pka[0:16, 36:100].rearrange("i (o k) -> i o k", k=2)
            w4t = pka[0:32, 100:132]
            fc1b = pkb[:, 0:1]
            fw2t = pkb[:, 1:3]
            fb2r = pkb[0:1, 3:5]

            # --- conv chain on PE + ScalarE ---
            v1 = sb.tile([8, 1], F32)
            nc.scalar.activation(v1[:], b1, ACT.Relu)

            w2s = sb.tile([8, 16], F32)
            nc.vector.tensor_reduce(out=w2s[:], in_=w2v, axis=X, op=ALU.add)
            p2 = ps.tile([16, 1], F32)
            nc.tensor.matmul(p2[:], w2s[:], v1[:], start=True, stop=True)
            v2 = sb.tile([16, 1], F32)
            nc.scalar.activation(v2[:], p2[:], ACT.Relu, bias=b2)

            w3s = sb.tile([16, 32], F32)
            nc.vector.tensor_reduce(out=w3s[:], in_=w3v, axis=X, op=ALU.add)
            p3 = ps.tile([32, 1], F32)
            nc.tensor.matmul(p3[:], w3s[:], v2[:], start=True, stop=True)
            v3 = sb.tile([32, 1], F32)
            nc.scalar.activation(v3[:], p3[:], ACT.Relu, bias=b3)

            p4 = ps.tile([32, 1], F32)
            nc.tensor.matmul(p4[:], w4t, v3[:], start=True, stop=True)
            v4 = sb.tile([32, 1], F32)
            nc.scalar.activation(v4[:], p4[:], ACT.Relu, bias=b4)

            # --- fc2 logit-difference prep (early, on DVE) ---
            dwp = sb.tile([128, 2], F32)
            nc.vector.tensor_tensor(
                out=dwp[:, 0:1], in0=fw2t[:, 0:1], in1=fw2t[:, 1:2], op=ALU.subtract
            )
            nc.vector.tensor_tensor(
                out=dwp[:, 1:2], in0=fw2t[:, 1:2], in1=fw2t[:, 0:1], op=ALU.subtract
            )
            dbp = sb.tile([1, 2], F32)
            nc.vector.tensor_tensor(
                out=dbp[:, 0:1], in0=fb2r[:, 0:1], in1=fb2r[:, 1:2], op=ALU.subtract
            )
            nc.vector.tensor_tensor(
                out=dbp[:, 1:2], in0=fb2r[:, 1:2], in1=fb2r[:, 0:1], op=ALU.subtract
            )

            # --- fc1: group-sum fc1_w over the 30 repeated positions (DVE,
            # chunked to overlap the DMA), PE-transpose, matvec on PE ---
            w1r = sb.tile([128, 32], F32)
            fw1v = fw1[:].rearrange("p (o t) -> p o t", t=30)
            for c in range(4):
                nc.vector.tensor_reduce(
                    out=w1r[:, c * 8 : (c + 1) * 8],
                    in_=fw1v[:, c * 8 : (c + 1) * 8],
                    axis=X,
                    op=ALU.add,
                )

            w1tp = ps.tile([32, 128], F32)
            nc.tensor.transpose(w1tp[:], w1r[:], ident[:])
            w1t = sb.tile([32, 128], F32)
            nc.scalar.activation(w1t[:], w1tp[:], ACT.Copy)

            py = ps.tile([128, 1], F32)
            nc.tensor.matmul(py[:], w1t[:], v4[:], start=True, stop=True)
            h = sb.tile([128, 1], F32)
            nc.scalar.activation(h[:], py[:], ACT.Relu, bias=fc1b)

            # --- fc2 logit difference + softmax(2) == sigmoid ---
            pl = ps.tile([1, 2], F32)
            nc.tensor.matmul(pl[:], h[:], dwp[:], start=True, stop=False)
            nc.tensor.matmul(pl[:], one[:1, :1], dbp[:], start=False, stop=True)

            probs = sb.tile([1, 2], F32)
            nc.scalar.activation(probs[:], pl[:], ACT.Sigmoid)
            nc.sync.dma_start(outd[:], probs[:])

    nc.compile()
    return nc


def _in_map(inputs):
    def f(name):
        return np.asarray(inputs[name], dtype=np.float32)

    pka = np.zeros((32, 132), dtype=np.float32)
    pka[0:8, 0] = f("conv1_b")
    pka[0:16, 1] = f("conv2_b")
    pka[0:32, 2] = f("conv3_b")
    pka[0:32, 3] = f("conv4_b")
    pka[0:8, 4:36] = f("conv2_w").transpose(1, 0, 2).reshape(8, 32)
    pka[0:16, 36:100] = f("conv3_w").transpose(1, 0, 2).reshape(16, 64)
    pka[0:32, 100:132] = f("conv4_w").reshape(32, 32).T

    pkb = np.zeros((128, 5), dtype=np.float32)
    pkb[:, 0] = f("fc1_b")
    pkb[:, 1:3] = f("fc2_w").T
    pkb[0, 3:5] = f("fc2_b")

    return {
        "pka": pka,
        "pkb": pkb,
        "ident": np.eye(128, dtype=np.float32),
        "fc1_w": np.ascontiguousarray(f("fc1_w")),
    }


def kernel(**inputs) -> np.ndarray:
    if "nc" not in _CACHE:
        _CACHE["nc"] = _build()
    nc = _CACHE["nc"]
    in_map = _in_map(inputs)
    res = run_bass_kernel_spmd(
        nc,
        [dict(in_map) for _ in range(N_CORES)],
        core_ids=list(range(N_CORES)),
    )
    return res.results[0]["out"].reshape(2).astype(np.float32)
